# revision 1
# baseline (speedup 1.0000x reference)
import sys

sys.path.insert(0, "/opt/trn_rl_repo")
import numpy as np

DIM = 1024
HEADS = 16
HD = 64
HID = 4096
EPS = 1e-5
NQ = 512          # queries per core
NK = 2048
P = 128
KC = DIM // P     # 8 c-chunks
KP = 4            # c-chunk pairs (DoubleRow K=256)
NKT = NK // P     # 16 kv chunks
QW = 256          # q-half width
S = 64.0          # fp8 weight/activation amplification
EXP_SCALE = 0.125 / (S * S)
OD = 1.0 / (S * S)   # oproj descale

_CACHE = {}


def _build():
    import concourse.bacc as bacc
    import concourse.tile as tile
    from concourse import mybir
    from concourse.masks import make_identity
    from contextlib import ExitStack

    F32 = mybir.dt.float32
    BF = mybir.dt.bfloat16
    F8 = mybir.dt.float8e4
    AF = mybir.ActivationFunctionType
    DR = mybir.MatmulPerfMode.DoubleRow

    nc = bacc.Bacc(None, target_bir_lowering=False, debug=False)

    tgt = nc.declare_dram_parameter("tgt", [P, 4, DIM], F32, isOutput=False)
    emb8 = nc.declare_dram_parameter("emb8", [P, KP, 2, NK], F8, isOutput=False)
    wq8 = nc.declare_dram_parameter("wq8", [P, 4, KP, 2, 2, P], F8, isOutput=False)
    wk8 = nc.declare_dram_parameter("wk8", [P, 4, KP, 2, 2, P], F8, isOutput=False)
    wv8 = nc.declare_dram_parameter("wv8", [P, KP, 2, 4, 256], F8, isOutput=False)
    wo8 = nc.declare_dram_parameter("wo8", [P, KP, 2, 8, P], F8, isOutput=False)
    # w1: [p, hg2(16), kc(8), 256]; w2: [p, mc(8), hc(32), 128]
    w1d = nc.declare_dram_parameter("w1", [P, 16, KC, 256], BF, isOutput=False)
    w2d = nc.declare_dram_parameter("w2", [P, 8, 32, P], BF, isOutput=False)
    bv = nc.declare_dram_parameter("bv", [DIM], F32, isOutput=False)
    # bias pack: [128, 64] = bq(8:Q*2+g) | bk(8) | bo(8) | b2(8) | b1(32)
    bias_pack = nc.declare_dram_parameter("bias_pack", [P, 64], F32, isOutput=False)
    out = nc.declare_dram_parameter("out", [P, 4, DIM], F32, isOutput=True)

    def bcast_ap(vec, n):
        import concourse.bass as bass
        return bass.AP(tensor=vec.tensor, offset=vec.offset, ap=[[0, P], [1, n]])

    def free_bcast_ap(t_ap, n0, rep):
        # [P, n0, rep] view of a [P, n0] slice, repeating along last dim
        import concourse.bass as bass
        return bass.AP(tensor=t_ap.tensor, offset=t_ap.offset,
                       ap=[list(t_ap.ap[0]), [t_ap.ap[1][0], n0], [0, rep]])

    def bv_pair_ap(bvb, q):
        # [P, 2, 256] view of bv_b[:, q*256:(q+1)*256] repeated along dim1
        import concourse.bass as bass
        return bass.AP(tensor=bvb.tensor, offset=bvb.offset + q * 256,
                       ap=[[bvb.ap[0][0], P], [0, 2], [1, 256]])

    with tile.TileContext(nc) as tc, ExitStack() as S_:
        const = S_.enter_context(tc.tile_pool(name="const", bufs=1))

        ident = const.tile([P, P], F32)
        make_identity(nc, ident)
        ident_r = const.tile([P, P], mybir.dt.float32r)
        nc.scalar.activation(ident_r[:], ident[:], AF.Copy)

        F32R = mybir.dt.float32r

        def tpr(out_ap, in_ap, start=True, stop=True):
            nc.tensor.matmul(out_ap.bitcast(F32R), in_ap, ident_r[:],
                             is_transpose=True, start=start, stop=stop,
                             skip_group_check=True)
        eps_t = const.tile([P, 1], F32)
        nc.vector.memset(eps_t[:], EPS)

        bv_b = const.tile([P, DIM], F32)
        nc.gpsimd.dma_start(out=bv_b[:], in_=bcast_ap(bv[:], DIM))

        bp = const.tile([P, 64], F32)
        nc.sync.dma_start(out=bp[:], in_=bias_pack[:, :])
        bq_s = bp[:, 0:8]
        bk_s = bp[:, 8:16]
        bo_s = bp[:, 16:24]
        b2_s = bp[:, 24:32]
        b1_s = bp[:, 32:64]

        tgt_sb = const.tile([P, 4, DIM], F32)
        nc.sync.dma_start(out=tgt_sb[:], in_=tgt[:, :, :])
        wo_sb = const.tile([P, KP, 2, 8, P], F8)
        nc.sync.dma_start(out=wo_sb[:], in_=wo8[:, :, :, :, :])

        persist = S_.enter_context(tc.tile_pool(name="persist", bufs=1))
        qT8 = [persist.tile([P, 2, NQ], F8, name=f"qT8_{q}") for q in range(4)]
        kT8 = [persist.tile([P, 2, NK], F8, name=f"kT8_{q}") for q in range(4)]
        v8 = [persist.tile([P, NKT, 4, 66], F8, name=f"v8_{q}") for q in range(4)]
        tgt2 = persist.tile([P, 4, DIM], F32)
        st2 = persist.tile([P, 4, 2, nc.vector.BN_STATS_DIM], F32)

        # ------- LN1 + lnT8 -------
        front_cm = tc.tile_pool(name="front", bufs=1, side="right")
        front = front_cm.__enter__()
        emb_sb = front.tile([P, KP, 2, NK], F8)
        nc.sync.dma_start(out=emb_sb[:], in_=emb8[:, :, :, :])
        wq_sb = front.tile([P, 4, KP, 2, 2, P], F8)
        nc.sync.dma_start(out=wq_sb[:], in_=wq8[:, :, :, :, :, :])
        wk_sb = front.tile([P, 4, KP, 2, 2, P], F8)
        nc.sync.dma_start(out=wk_sb[:], in_=wk8[:, :, :, :, :, :])
        wv_sb = front.tile([P, KP, 2, 4, 256], F8)
        nc.sync.dma_start(out=wv_sb[:], in_=wv8[:, :, :, :, :])
        lnT8 = front.tile([P, KP, 2, NQ], F8)

        ln1_cm = tc.tile_pool(name="lnw", bufs=4, side="right")
        lnw = ln1_cm.__enter__()
        ln1p_cm = tc.tile_pool(name="fp_ps", bufs=4, space="PSUM", side="right")
        fp_ps = ln1p_cm.__enter__()
        ln_t = [lnw.tile([P, DIM], F32R, name=f"ln{t}", tag=f"ln{t}") for t in range(4)]
        for t in range(4):
            stt = lnw.tile([P, 2, nc.vector.BN_STATS_DIM], F32, name=f"st{t}", tag="st")
            for sg in range(2):
                nc.vector.bn_stats(out=stt[:, sg, :],
                                   in_=tgt_sb[:, t, sg * 512:(sg + 1) * 512])
            mv = lnw.tile([P, nc.vector.BN_AGGR_DIM], F32, name=f"mv{t}", tag="mv")
            nc.vector.bn_aggr(out=mv[:], in_=stt[:])
            rstd = lnw.tile([P, 1], F32, name=f"rstd{t}", tag="rstd")
            nc.scalar.activation(out=rstd[:], in_=mv[:, 1:2], func=AF.Sqrt,
                                 bias=eps_t[:], scale=1.0)
            nc.vector.reciprocal(out=rstd[:], in_=rstd[:])
            nc.vector.tensor_scalar(out=ln_t[t][:], in0=tgt_sb[:, t, :],
                                    scalar1=mv[:, 0:1], scalar2=rstd[:],
                                    op0=mybir.AluOpType.subtract,
                                    op1=mybir.AluOpType.mult)
        for t in range(4):
            for kc2 in range(KC // 2):
                pt = fp_ps.tile([P, 512], F32, name="pt", tag="tp")
                for j in range(2):
                    tpr(pt[:, j * P:(j + 1) * P],
                        ln_t[t][:, (kc2 * 2 + j) * P:(kc2 * 2 + j + 1) * P],
                        start=(j == 0), stop=(j == 1))
                nc.vector.tensor_copy(
                    lnT8[:, kc2, :, t * P:(t + 1) * P],
                    pt[:, 0:256].rearrange("p (two c) -> p two c", two=2))
        ln1p_cm.__exit__(None, None, None)
        ln1_cm.__exit__(None, None, None)

        # front psum pools (coexist with attention pools)
        pjp_cm = tc.tile_pool(name="pj_ps", bufs=1, space="PSUM", side="right")
        pj_ps = pjp_cm.__enter__()
        vpp_cm = tc.tile_pool(name="vp_ps", bufs=1, space="PSUM", side="right")
        vp_ps = vpp_cm.__enter__()

        def emit_front(q):
            ps = pj_ps.tile([P, 2, NQ], F32, name="qps", tag="pj")
            for g in range(2):
                for kp in range(KP):
                    nc.tensor.matmul(ps[:, g, :], wq_sb[:, q, kp, :, g, :],
                                     lnT8[:, kp, :, :],
                                     start=(kp == 0), stop=(kp == KP - 1),
                                     perf_mode=DR)
            nc.vector.tensor_tensor(out=qT8[q][:], in0=ps[:],
                                    in1=free_bcast_ap(bq_s[:, q * 2:q * 2 + 2], 2, NQ),
                                    op=mybir.AluOpType.add)
            for kv4 in range(4):
                ps = pj_ps.tile([P, 2, NQ], F32, name="kps", tag="pj")
                for g in range(2):
                    for kp in range(KP):
                        nc.tensor.matmul(ps[:, g, :], wk_sb[:, q, kp, :, g, :],
                                         emb_sb[:, kp, :, kv4 * 512:(kv4 + 1) * 512],
                                         start=(kp == 0), stop=(kp == KP - 1),
                                         perf_mode=DR)
                nc.vector.tensor_tensor(out=kT8[q][:, :, kv4 * 512:(kv4 + 1) * 512],
                                        in0=ps[:],
                                        in1=free_bcast_ap(bk_s[:, q * 2:q * 2 + 2], 2, 512),
                                        op=mybir.AluOpType.add)
            nc.vector.memset(v8[q][:, :, :, 0], 1.0)
            for tpair in range(NKT // 2):
                ps = vp_ps.tile([P, 2, 256], F32, name="vps", tag="vp")
                for j in range(2):
                    kvt = tpair * 2 + j
                    for kp in range(KP):
                        nc.tensor.matmul(ps[:, j, :],
                                         emb_sb[:, kp, :, kvt * P:(kvt + 1) * P],
                                         wv_sb[:, kp, :, q, :],
                                         start=(j == 0 and kp == 0),
                                         stop=(j == 1 and kp == KP - 1),
                                         perf_mode=DR, skip_group_check=True)
                nc.vector.tensor_tensor(
                    out=v8[q][:, tpair * 2:tpair * 2 + 2, :, 1:65],
                    in0=ps[:].rearrange("p two (h d) -> p two h d", d=64),
                    in1=bv_pair_ap(bv_b, q),
                    op=mybir.AluOpType.add)

        # ------- attention + mlp pools -------
        attn = S_.enter_context(tc.tile_pool(name="attn", bufs=1))
        ctxq = [attn.tile([P, 2, DIM], F32R, name=f"ctxq{h}") for h in range(2)]
        scp = S_.enter_context(tc.tile_pool(name="scp", bufs=1, space="PSUM"))
        scp2 = S_.enter_context(tc.tile_pool(name="scp2", bufs=1, space="PSUM"))
        cpp = S_.enter_context(tc.tile_pool(name="cpp", bufs=1, space="PSUM"))
        exp_pool = S_.enter_context(tc.tile_pool(name="exp", bufs=3))
        axw = S_.enter_context(tc.tile_pool(name="axw", bufs=2))
        heads = [(q, hp) for q in range(4) for hp in range(4)]
        ex_tiles = {}

        def emit_scores_exp(hd, half):
            q, hp = hd
            ex8 = exp_pool.tile([P, NKT, QW], F8, name=f"ex_{half}_{q}_{hp}", tag="ex")
            ex_tiles[(q, hp, half)] = ex8
            for grp in range(4):
                pool = scp if grp % 2 == 0 else scp2
                sc = pool.tile([P, 4, QW], F32, name="sc", tag=f"sc{grp % 2}")
                for j in range(4):
                    kvt = grp * 4 + j
                    nc.tensor.matmul(
                        sc[:, j, :],
                        kT8[q][hp * 32:(hp + 1) * 32, :, kvt * P:(kvt + 1) * P],
                        qT8[q][hp * 32:(hp + 1) * 32, :, half * QW:(half + 1) * QW],
                        start=(j % 2 == 0), stop=(j % 2 == 1), perf_mode=DR,
                        tile_position=(hp * 32, 0), skip_group_check=True)
                nc.scalar.activation(ex8[:, grp * 4:(grp + 1) * 4, :], sc[:],
                                     AF.Exp, scale=EXP_SCALE)

        def emit_ctx(hd, half):
            q, hp = hd
            hg = q * 4 + hp
            ex8 = ex_tiles.pop((q, hp, half))
            cps = cpp.tile([P, 2, QW], F32, name="cps", tag="cps")
            for qt in range(2):
                for t in range(NKT // 2):
                    nc.tensor.matmul(
                        cps[:, qt, 0:65],
                        ex8[:, t * 2:t * 2 + 2, qt * P:(qt + 1) * P],
                        v8[q][:, t * 2:t * 2 + 2, hp, 0:65],
                        start=(qt == 0 and t == 0),
                        stop=(qt == 1 and t == NKT // 2 - 1),
                        perf_mode=DR, skip_group_check=True)
            rcp = axw.tile([P, 2, 1], F32, name="rcp", tag="rcp")
            nc.vector.reciprocal(out=rcp[:], in_=cps[:, :, 0:1])
            nc.vector.tensor_tensor(out=ctxq[half][:, :, hg * HD:(hg + 1) * HD],
                                    in0=cps[:, :, 1:65],
                                    in1=free_bcast_ap(rcp[:, :, 0], 2, HD),
                                    op=mybir.AluOpType.mult)

        def build_mlp_thunks(half, tp_ps, mm_ps):
            th = []
            ctxT8h = attn.tile([P, KP, 2, QW], F8, name=f"ctxT8_{half}", tag="ctxT8")
            ln2_t = attn.tile([P, 2, DIM], F32R, name=f"ln2_{half}", tag="ln2")
            ln2T = attn.tile([P, KC, QW], BF, name=f"ln2T_{half}", tag="ln2T")
            h1T = mlp.tile([P, 32, QW], BF, name=f"h1T_{half}", tag="h1T")
            w1_tiles = {}
            w2_tiles = {}

            def tp_ctx(qt, kc2):
                def f():
                    pt = tp_ps.tile([P, 512], F32, name="tpt", tag="tp")
                    for j in range(2):
                        tpr(pt[:, j * P:(j + 1) * P],
                            ctxq[half][:, qt, (kc2 * 2 + j) * P:(kc2 * 2 + j + 1) * P],
                            start=(j == 0), stop=(j == 1))
                    nc.vector.tensor_copy(
                        ctxT8h[:, kc2, :, qt * P:(qt + 1) * P],
                        pt[:, 0:256].rearrange("p (two c) -> p two c", two=2))
                return f
            for qt in range(2):
                for kc2 in range(KC // 2):
                    th.append(tp_ctx(qt, kc2))

            def oproj(mc):
                def f():
                    ps = mm_ps.tile([P, 512], F32, name="ops", tag="mm")
                    for kp in range(KP):
                        nc.tensor.matmul(ps[:, 0:QW], wo_sb[:, kp, :, mc, :],
                                         ctxT8h[:, kp, :, :],
                                         start=(kp == 0), stop=(kp == KP - 1),
                                         perf_mode=DR)
                    yt = mlw.tile([P, QW], F32R, name="yt", tag="yt")
                    nc.vector.tensor_scalar(out=yt[:], in0=ps[:, 0:QW],
                                            scalar1=OD, scalar2=bo_s[:, mc:mc + 1],
                                            op0=mybir.AluOpType.mult,
                                            op1=mybir.AluOpType.add)
                    pt = tp_ps.tile([P, 512], F32, name="tpt", tag="tp")
                    for qt in range(2):
                        tpr(pt[:, qt * P:(qt + 1) * P], yt[:, qt * P:(qt + 1) * P],
                            start=(qt == 0), stop=(qt == 1))
                    nc.vector.tensor_tensor(
                        out=tgt2[:, half * 2:half * 2 + 2, mc * P:(mc + 1) * P],
                        in0=pt[:, 0:256].rearrange("p (two c) -> p two c", two=2),
                        in1=tgt_sb[:, half * 2:half * 2 + 2, mc * P:(mc + 1) * P],
                        op=mybir.AluOpType.add)
                return f
            for mc in range(8):
                th.append(oproj(mc))

            def ln2_stats(qt):
                def f():
                    gt = half * 2 + qt
                    for sg in range(2):
                        nc.vector.bn_stats(out=st2[:, gt, sg, :],
                                           in_=tgt2[:, gt, sg * 512:(sg + 1) * 512])
                    mv = mlw.tile([P, nc.vector.BN_AGGR_DIM], F32, name="mv2", tag="mv2")
                    nc.vector.bn_aggr(out=mv[:], in_=st2[:, gt, :, :])
                    rstd = mlw.tile([P, 1], F32, name="rstd2", tag="rstd2")
                    nc.scalar.activation(out=rstd[:], in_=mv[:, 1:2], func=AF.Sqrt,
                                         bias=eps_t[:], scale=1.0)
                    nc.vector.reciprocal(out=rstd[:], in_=rstd[:])
                    nc.vector.tensor_scalar(out=ln2_t[:, qt, :], in0=tgt2[:, gt, :],
                                            scalar1=mv[:, 0:1], scalar2=rstd[:],
                                            op0=mybir.AluOpType.subtract,
                                            op1=mybir.AluOpType.mult)
                return f
            def ln2_tp(qt, kc2):
                def f():
                    pt = tp_ps.tile([P, 512], F32, name="tpt", tag="tp")
                    for j in range(2):
                        tpr(pt[:, j * P:(j + 1) * P],
                            ln2_t[:, qt, (kc2 * 2 + j) * P:(kc2 * 2 + j + 1) * P],
                            start=(j == 0), stop=(j == 1))
                    nc.vector.tensor_copy(
                        ln2T[:, kc2 * 2:kc2 * 2 + 2, qt * P:(qt + 1) * P],
                        pt[:, 0:256].rearrange("p (two c) -> p two c", two=2))
                return f
            for qt in range(2):
                th.append(ln2_stats(qt))
                for kc2 in range(KC // 2):
                    th.append(ln2_tp(qt, kc2))

            def w1_load(hg2):
                wt = w1p.tile([P, KC, QW], BF, name=f"w1c{hg2 % 2}", tag="w1c")
                nc.sync.dma_start(out=wt[:], in_=w1d[:, hg2, :, :])
                w1_tiles[hg2] = wt

            def fc1(hg2):
                def f():
                    if hg2 == 0:
                        w1_load(0)
                    if hg2 + 1 < 16:
                        w1_load(hg2 + 1)
                    wt = w1_tiles.pop(hg2)
                    for j in range(2):
                        ps = mm_ps.tile([P, 512], F32, name="f1p", tag="mm")
                        for kc in range(KC):
                            nc.tensor.matmul(ps[:, 0:QW],
                                             wt[:, kc, j * P:(j + 1) * P],
                                             ln2T[:, kc, :],
                                             start=(kc == 0), stop=(kc == KC - 1))
                        hm = hg2 * 2 + j
                        nc.scalar.activation(h1T[:, hm, :], ps[:, 0:QW], AF.Gelu,
                                             bias=b1_s[:, hm:hm + 1])
                return f
            for hg2 in range(16):
                th.append(fc1(hg2))

            def w2_load(mc):
                wt = w2p.tile([P, 32, P], BF, name=f"w2c{mc % 2}", tag="w2c")
                nc.sync.dma_start(out=wt[:], in_=w2d[:, mc, :, :])
                w2_tiles[mc] = wt

            def fc2(mc):
                def f():
                    if mc == 0:
                        w2_load(0)
                    if mc + 1 < 8:
                        w2_load(mc + 1)
                    wt = w2_tiles.pop(mc)
                    ps = mm_ps.tile([P, 512], F32, name="f2p", tag="mm")
                    for hc in range(32):
                        nc.tensor.matmul(ps[:, 0:QW], wt[:, hc, :], h1T[:, hc, :],
                                         start=(hc == 0), stop=(hc == 31))
                    y2 = mlw.tile([P, QW], F32R, name="y2", tag="y2")
                    nc.vector.tensor_scalar_add(y2[:], ps[:, 0:QW], b2_s[:, mc:mc + 1])
                    pt = tp_ps.tile([P, 512], F32, name="tpt", tag="tp")
                    for qt in range(2):
                        tpr(pt[:, qt * P:(qt + 1) * P], y2[:, qt * P:(qt + 1) * P],
                            start=(qt == 0), stop=(qt == 1))
                    nc.vector.tensor_tensor(
                        out=tgt2[:, half * 2:half * 2 + 2, mc * P:(mc + 1) * P],
                        in0=pt[:, 0:256].rearrange("p (two c) -> p two c", two=2),
                        in1=tgt2[:, half * 2:half * 2 + 2, mc * P:(mc + 1) * P],
                        op=mybir.AluOpType.add)
                return f
            for mc in range(8):
                th.append(fc2(mc))

            def store():
                for qt in range(2):
                    gt = half * 2 + qt
                    nc.sync.dma_start(out=out[:, gt, :], in_=tgt2[:, gt, :])
            th.append(store)
            return th

        # ---- half 0: front per quarter + heads (ctx lag 1) ----
        done = []
        for q in range(4):
            emit_front(q)
            for hp in range(4):
                emit_scores_exp((q, hp), 0)
                done.append((q, hp))
                if len(done) >= 2:
                    emit_ctx(done[-2], 0)
        emit_ctx(done[-1], 0)
        vpp_cm.__exit__(None, None, None)
        pjp_cm.__exit__(None, None, None)
        front_cm.__exit__(None, None, None)

        tp_ps = S_.enter_context(tc.tile_pool(name="tp_ps", bufs=1, space="PSUM"))
        mm_ps = S_.enter_context(tc.tile_pool(name="mm_ps", bufs=2, space="PSUM"))
        mlp = S_.enter_context(tc.tile_pool(name="mlp", bufs=1))
        w1p = S_.enter_context(tc.tile_pool(name="w1p", bufs=2))
        w2p = S_.enter_context(tc.tile_pool(name="w2p", bufs=2))
        mlw = S_.enter_context(tc.tile_pool(name="mlw", bufs=4))

        # ---- mlp0 interleaved with half-1 heads (ctx lag 2) ----
        mlp0 = build_mlp_thunks(0, tp_ps, mm_ps)
        nch = (len(mlp0) + 15) // 16
        for i, hd in enumerate(heads):
            emit_scores_exp(hd, 1)
            for t in mlp0[i * nch:(i + 1) * nch]:
                t()
            if i >= 2:
                emit_ctx(heads[i - 2], 1)
        for t in mlp0[16 * nch:]:
            t()
        emit_ctx(heads[14], 1)
        emit_ctx(heads[15], 1)

        for t in build_mlp_thunks(1, tp_ps, mm_ps):
            t()

    nc.compile()
    return nc


def _get_nc():
    if "nc" not in _CACHE:
        _CACHE["nc"] = _build()
    return _CACHE["nc"]


def kernel(tgt, emb_motion, ln_g, ln_b, wq, bq, wk, bk, wv, bv, wo, bo, w1, b1, w2, b2):
    from concourse.bass_utils import run_bass_kernel_spmd
    import ml_dtypes

    nc = _get_nc()
    f = np.ascontiguousarray
    a32 = lambda x: np.asarray(x, np.float32)
    F8 = ml_dtypes.float8_e4m3
    BF16 = ml_dtypes.bfloat16

    g32, b32 = a32(ln_g), a32(ln_b)
    wq_e = a32(wq) * g32[:, None]
    bq_e = a32(bq) + b32 @ a32(wq)
    w1_e = a32(w1) * g32[:, None]
    b1_e = a32(b1) + b32 @ a32(w1)

    Sv = np.float32(S)

    # wq8/wk8: [c,d] -> [p, Q, kp, kt, g, h'*32+r]
    def qk_tile(w):
        arr = np.asarray(w * Sv, F8)
        arr = arr.reshape(4, 2, 128, 4, 4, 2, 32)      # [kp,kt,p, Q,h',g,r]
        arr = arr.transpose(2, 3, 0, 1, 5, 4, 6)       # [p, Q, kp, kt, g, h', r]
        return f(arr.reshape(128, 4, 4, 2, 2, 128))

    wq_t = qk_tile(wq_e)
    wk_t = qk_tile(a32(wk))
    wv_t = f(np.asarray(a32(wv) * Sv, F8).reshape(4, 2, 128, 4, 256).transpose(2, 0, 1, 3, 4))
    wo_t = f(np.asarray(a32(wo) * Sv, F8).reshape(4, 2, 128, 8, 128).transpose(2, 0, 1, 3, 4))
    # w1: [c,hid] -> [p, hg2, kc, 256]
    w1_t = f(np.asarray(w1_e, BF16).reshape(8, 128, 16, 256).transpose(1, 2, 0, 3))
    # w2: [hc*128+p, d] -> [p, mc, hc, 128]
    w2_t = f(np.asarray(a32(w2), BF16).reshape(32, 128, 8, 128).transpose(1, 2, 0, 3))

    def qk_bias(b):
        r = (b * Sv).reshape(4, 4, 2, 32)              # [Q,h',g,r]
        return r.transpose(0, 2, 1, 3).reshape(8, 128).T
    bias_pack = np.concatenate([
        qk_bias(bq_e), qk_bias(a32(bk)),
        a32(bo).reshape(8, 128).T, a32(b2).reshape(8, 128).T,
        b1_e.reshape(32, 128).T,
    ], axis=1)
    bias_pack = f(bias_pack.astype(np.float32))
    bv_s = f((a32(bv) * Sv).astype(np.float32))

    B = tgt.shape[0]
    in_maps = []
    for c in range(8):
        b, h = divmod(c, 2)
        tgt_c = a32(tgt[b, h * NQ:(h + 1) * NQ])
        tgt_t = f(tgt_c.reshape(4, 128, DIM).transpose(1, 0, 2))
        emb_t = np.asarray(a32(emb_motion[b]).T, F8)
        emb_t = f(emb_t.reshape(4, 2, 128, NK).transpose(2, 0, 1, 3))
        in_maps.append({
            "tgt": tgt_t, "emb8": emb_t,
            "wq8": wq_t, "wk8": wk_t, "wv8": wv_t, "wo8": wo_t,
            "w1": w1_t, "w2": w2_t,
            "bv": bv_s, "bias_pack": bias_pack,
        })
    r = run_bass_kernel_spmd(nc, in_maps, list(range(8)))
    res = np.empty((B, 1024, DIM), np.float32)
    for c in range(8):
        b, h = divmod(c, 2)
        res[b, h * NQ:(h + 1) * NQ] = r.results[c]["out"].transpose(1, 0, 2).reshape(NQ, DIM)
    return res



# revision 30
# speedup vs baseline: 1.1904x; 1.1904x over previous
import sys

sys.path.insert(0, "/opt/trn_rl_repo")
import numpy as np

DIM = 1024
HEADS = 16
HD = 64
HID = 4096
EPS = 1e-5
NQ = 512          # queries per core
NK = 2048
P = 128
KC = DIM // P     # 8 c-chunks
KP = 4            # c-chunk pairs (DoubleRow K=256)
NKT = NK // P     # 16 kv chunks
QW = 256          # q-half width
S = 64.0          # fp8 weight/activation amplification
EXP_SCALE = 0.125 / (S * S)
OD = 1.0 / (S * S)   # oproj descale
OD2 = 1.0          # fc2 is bf16, no descale

_CACHE = {}


def _build():
    import concourse.bacc as bacc
    import concourse.tile as tile
    from concourse import mybir
    from concourse.masks import make_identity
    from contextlib import ExitStack

    F32 = mybir.dt.float32
    BF = mybir.dt.bfloat16
    F8 = mybir.dt.float8e4
    AF = mybir.ActivationFunctionType
    DR = mybir.MatmulPerfMode.DoubleRow

    nc = bacc.Bacc(None, target_bir_lowering=False, debug=False)

    tgt = nc.declare_dram_parameter("tgt", [P, 4, DIM], F32, isOutput=False)
    emb8 = nc.declare_dram_parameter("emb8", [P, KP, 2, NK], F8, isOutput=False)
    wq8 = nc.declare_dram_parameter("wq8", [P, 4, KP, 2, 2, P], F8, isOutput=False)
    wk8 = nc.declare_dram_parameter("wk8", [P, 4, KP, 2, 2, P], F8, isOutput=False)
    wv8 = nc.declare_dram_parameter("wv8", [P, KP, 2, 4, 256], F8, isOutput=False)
    wo8 = nc.declare_dram_parameter("wo8", [P, KP, 2, 8, P], F8, isOutput=False)
    # w1: [p, hm(32), kp(4), 2, 128] fp8 ; w2: [p, mc(8), hp(16), 2, 128] fp8
    w1d = nc.declare_dram_parameter("w1", [P, 32, KP, 2, P], F8, isOutput=False)
    w2d = nc.declare_dram_parameter("w2", [P, 8, 32, P], BF, isOutput=False)
    # bias pack: [128, 56] = bq(8:Q*2+g) | bo(8) | b2(8) | b1(32)
    bias_pack = nc.declare_dram_parameter("bias_pack", [P, 56], F32, isOutput=False)
    out = nc.declare_dram_parameter("out", [P, 4, DIM], F32, isOutput=True)

    def bcast_ap(vec, n):
        import concourse.bass as bass
        return bass.AP(tensor=vec.tensor, offset=vec.offset, ap=[[0, P], [1, n]])

    def free_bcast_ap(t_ap, n0, rep):
        # [P, n0, rep] view of a [P, n0] slice, repeating along last dim
        import concourse.bass as bass
        return bass.AP(tensor=t_ap.tensor, offset=t_ap.offset,
                       ap=[list(t_ap.ap[0]), [t_ap.ap[1][0], n0], [0, rep]])

    with tile.TileContext(nc) as tc, ExitStack() as S_:
        const = S_.enter_context(tc.tile_pool(name="const", bufs=1))

        ident = const.tile([P, P], F32)
        make_identity(nc, ident)
        ident_b = const.tile([P, P], BF)
        nc.scalar.activation(ident_b[:], ident[:], AF.Copy)

        def tpb(out_ap, in_ap, start=True, stop=True):
            # bf16 transpose: out (psum, bf16) = in_ap^T
            nc.tensor.matmul(out_ap, in_ap, ident_b[:],
                             is_transpose=True, start=start, stop=stop,
                             skip_group_check=True)

        eps_t = const.tile([P, 1], F32)
        nc.vector.memset(eps_t[:], EPS)

        bp = const.tile([P, 56], F32)
        nc.sync.dma_start(out=bp[:], in_=bias_pack[:, :])
        bq_s = bp[:, 0:8]
        bo_s = bp[:, 8:16]
        b2_s = bp[:, 16:24]
        b1_s = bp[:, 24:56]

        tgt_sb = const.tile([P, 4, DIM], F32)
        for t in range(4):
            nc.sync.dma_start(out=tgt_sb[:, t, :], in_=tgt[:, t, :])
        wo_sb = const.tile([P, KP, 2, 8, P], F8)  # loaded after front weights

        persist = S_.enter_context(tc.tile_pool(name="persist", bufs=1))
        qT8 = [persist.tile([P, 2, NQ], F8, name=f"qT8_{q}") for q in range(4)]
        kT8 = [persist.tile([P, 2, NK], F8, name=f"kT8_{q}") for q in range(4)]
        v8 = [persist.tile([P, NKT, 4, 66], F8, name=f"v8_{q}") for q in range(4)]
        tgt2 = persist.tile([P, 4, DIM], F32)
        st2 = persist.tile([P, 4, 2, nc.vector.BN_STATS_DIM], F32)

        def rstd_from_var(rstd_ap, var_ap, scratch_pool, tag):
            # rstd = exp(-0.5*ln(var+eps)); Ln+Exp share an act table
            lnv = scratch_pool.tile([P, 1], F32, name="lnv", tag=tag)
            nc.scalar.activation(out=lnv[:], in_=var_ap, func=AF.Ln,
                                 bias=eps_t[:], scale=1.0)
            nc.scalar.activation(out=rstd_ap, in_=lnv[:], func=AF.Exp,
                                 scale=-0.5)

        # ------- LN1 + lnT8 -------
        front_cm = tc.tile_pool(name="front", bufs=1, side="right")
        front = front_cm.__enter__()
        emb_sb = front.tile([P, KP, 2, NK], F8)
        for h in range(2):
            nc.sync.dma_start(out=emb_sb[:, :, :, h * 1024:(h + 1) * 1024],
                              in_=emb8[:, :, :, h * 1024:(h + 1) * 1024])
        wk_sb = front.tile([P, 4, KP, 2, 2, P], F8)
        nc.sync.dma_start(out=wk_sb[:], in_=wk8[:, :, :, :, :, :])
        wq_sb = front.tile([P, 4, KP, 2, 2, P], F8)
        nc.sync.dma_start(out=wq_sb[:], in_=wq8[:, :, :, :, :, :])
        wv_sb = front.tile([P, KP, 2, 4, 256], F8)
        nc.sync.dma_start(out=wv_sb[:], in_=wv8[:, :, :, :, :])
        nc.sync.dma_start(out=wo_sb[:], in_=wo8[:, :, :, :, :])
        lnT8 = front.tile([P, KP, 2, NQ], F8)

        # K-projection for q=0 first: independent of LN1, unblocks the first
        # scores/exp as soon as emb+wk arrive
        kp0_cm = tc.tile_pool(name="kp0_ps", bufs=1, space="PSUM", side="right")
        kp0_ps = kp0_cm.__enter__()

        def emit_kproj(q, pool, on_act):
            for kv4 in range(4):
                ps = pool.tile([P, 2, NQ], F32, name="kps", tag="pj")
                for g in range(2):
                    for kp in range(KP):
                        nc.tensor.matmul(ps[:, g, :], wk_sb[:, q, kp, :, g, :],
                                         emb_sb[:, kp, :, kv4 * 512:(kv4 + 1) * 512],
                                         start=(kp == 0), stop=(kp == KP - 1),
                                         perf_mode=DR)
                if on_act:
                    nc.scalar.activation(kT8[q][:, :, kv4 * 512:(kv4 + 1) * 512],
                                         ps[:], AF.Copy)
                else:
                    nc.vector.tensor_copy(out=kT8[q][:, :, kv4 * 512:(kv4 + 1) * 512],
                                          in_=ps[:])

        emit_kproj(0, kp0_ps, True)

        ln1_cm = tc.tile_pool(name="lnw", bufs=4, side="right")
        lnw = ln1_cm.__enter__()
        ln1p_cm = tc.tile_pool(name="fp_ps", bufs=2, space="PSUM", side="right")
        fp_ps = ln1p_cm.__enter__()
        ln_t = [lnw.tile([P, DIM], BF, name=f"ln{t}", tag=f"ln{t}") for t in range(4)]
        mv4 = lnw.tile([P, 4, nc.vector.BN_AGGR_DIM], F32, name="mv4", tag="mv")
        for t in range(4):
            stt = lnw.tile([P, 2, nc.vector.BN_STATS_DIM], F32, name=f"st{t}", tag="st")
            for sg in range(2):
                nc.vector.bn_stats(out=stt[:, sg, :],
                                   in_=tgt_sb[:, t, sg * 512:(sg + 1) * 512])
            nc.vector.bn_aggr(out=mv4[:, t, :], in_=stt[:])
        # batched rstd for all 4 tiles: one Ln + one Exp (same act table)
        lnv4 = lnw.tile([P, 4], F32, name="lnv4", tag="lnv4")
        nc.scalar.activation(out=lnv4[:], in_=mv4[:, :, 1], func=AF.Ln,
                             bias=eps_t[:], scale=1.0)
        rstd4 = lnw.tile([P, 4], F32, name="rstd4", tag="rstd4")
        nc.scalar.activation(out=rstd4[:], in_=lnv4[:], func=AF.Exp, scale=-0.5)
        for t in range(4):
            nc.vector.tensor_scalar(out=ln_t[t][:], in0=tgt_sb[:, t, :],
                                    scalar1=mv4[:, t, 0:1], scalar2=rstd4[:, t:t + 1],
                                    op0=mybir.AluOpType.subtract,
                                    op1=mybir.AluOpType.mult)
        for t in range(4):
            pt = fp_ps.tile([P, KP, 2, P], BF, name="pt", tag="tp")
            for kc in range(KC):
                tpb(pt[:, kc // 2, kc % 2, :],
                    ln_t[t][:, kc * P:(kc + 1) * P],
                    start=(kc == 0), stop=(kc == KC - 1))
            # ACT copy (Copy is in every act table): DVE is the lead-in
            # bottleneck, ACT is idle here
            nc.scalar.activation(lnT8[:, :, :, t * P:(t + 1) * P], pt[:], AF.Copy)
        ln1p_cm.__exit__(None, None, None)
        ln1_cm.__exit__(None, None, None)
        kp0_cm.__exit__(None, None, None)

        # attention psum pools (right stack, below pj/vp so that pj/vp can be
        # released at end of half-0 and scp/cpp later, both in LIFO order)
        scp_cm = tc.tile_pool(name="scp", bufs=2, space="PSUM", side="right")
        scp = scp_cm.__enter__()
        cpp_cm = tc.tile_pool(name="cpp", bufs=1, space="PSUM", side="right")
        cpp = cpp_cm.__enter__()

        # front psum pools (coexist with attention pools)
        pjp_cm = tc.tile_pool(name="pj_ps", bufs=1, space="PSUM", side="right")
        pj_ps = pjp_cm.__enter__()
        vpp_cm = tc.tile_pool(name="vp_ps", bufs=1, space="PSUM", side="right")
        vp_ps = vpp_cm.__enter__()

        def emit_front(q):
            # q=0,1: psum->sbuf moves on ACT (idle during lead-in, and Copy
            # never causes act-table loads); q=2,3: on DVE (ACT saturated by
            # then with exps)
            on_act = q < 2
            if q > 0:
                emit_kproj(q, pj_ps, on_act)
            ps = pj_ps.tile([P, 2, NQ], F32, name="qps", tag="pj")
            for g in range(2):
                for kp in range(KP):
                    nc.tensor.matmul(ps[:, g, :], wq_sb[:, q, kp, :, g, :],
                                     lnT8[:, kp, :, :],
                                     start=(kp == 0), stop=(kp == KP - 1),
                                     perf_mode=DR)
            if on_act:
                for g in range(2):
                    nc.scalar.activation(qT8[q][:, g, :], ps[:, g, :], AF.Identity,
                                         bias=bq_s[:, q * 2 + g:q * 2 + g + 1])
            else:
                nc.vector.tensor_tensor(out=qT8[q][:], in0=ps[:],
                                        in1=free_bcast_ap(bq_s[:, q * 2:q * 2 + 2], 2, NQ),
                                        op=mybir.AluOpType.add)
            nc.vector.memset(v8[q][:, :, :, 0], 1.0)
            for tpair in range(NKT // 2):
                ps = vp_ps.tile([P, 2, 256], F32, name="vps", tag="vp")
                for j in range(2):
                    kvt = tpair * 2 + j
                    for kp in range(KP):
                        nc.tensor.matmul(ps[:, j, :],
                                         emb_sb[:, kp, :, kvt * P:(kvt + 1) * P],
                                         wv_sb[:, kp, :, q, :],
                                         start=(j == 0 and kp == 0),
                                         stop=(j == 1 and kp == KP - 1),
                                         perf_mode=DR, skip_group_check=True)
                nc.vector.tensor_copy(
                    out=v8[q][:, tpair * 2:tpair * 2 + 2, :, 1:65],
                    in_=ps[:].rearrange("p two (h d) -> p two h d", d=64))

        # ------- attention + mlp pools -------
        attn = S_.enter_context(tc.tile_pool(name="attn", bufs=1))
        ctxq = [attn.tile([P, 2, DIM], BF, name=f"ctxq{h}") for h in range(2)]
        exp_pool = S_.enter_context(tc.tile_pool(name="exp", bufs=3))
        axw = S_.enter_context(tc.tile_pool(name="axw", bufs=2))
        heads = [(q, hp) for q in range(4) for hp in range(4)]
        ex_tiles = {}

        def emit_scores_exp(hd, half):
            q, hp = hd
            ex8 = exp_pool.tile([P, NKT, QW], F8, name=f"ex_{half}_{q}_{hp}", tag="ex")
            ex_tiles[(q, hp, half)] = ex8
            for grp in range(4):
                sc = scp.tile([P, 4, QW], F32, name="sc", tag="sc")
                for j in range(4):
                    kvt = grp * 4 + j
                    nc.tensor.matmul(
                        sc[:, j, :],
                        kT8[q][hp * 32:(hp + 1) * 32, :, kvt * P:(kvt + 1) * P],
                        qT8[q][hp * 32:(hp + 1) * 32, :, half * QW:(half + 1) * QW],
                        start=(j % 2 == 0), stop=(j % 2 == 1), perf_mode=DR,
                        tile_position=(hp * 32, 0), skip_group_check=True)
                nc.scalar.activation(ex8[:, grp * 4:(grp + 1) * 4, :], sc[:],
                                     AF.Exp, scale=EXP_SCALE)

        def emit_ctx(hd, half):
            q, hp = hd
            hg = q * 4 + hp
            ex8 = ex_tiles.pop((q, hp, half))
            cps = cpp.tile([P, 2, QW], F32, name="cps", tag="cps")
            for qt in range(2):
                for t in range(NKT // 2):
                    nc.tensor.matmul(
                        cps[:, qt, 0:65],
                        ex8[:, t * 2:t * 2 + 2, qt * P:(qt + 1) * P],
                        v8[q][:, t * 2:t * 2 + 2, hp, 0:65],
                        start=(qt == 0 and t == 0),
                        stop=(qt == 1 and t == NKT // 2 - 1),
                        perf_mode=DR, skip_group_check=True)
            rcp = axw.tile([P, 2, 1], F32, name="rcp", tag="rcp")
            nc.vector.reciprocal(out=rcp[:], in_=cps[:, :, 0:1])
            nc.vector.tensor_tensor(out=ctxq[half][:, :, hg * HD:(hg + 1) * HD],
                                    in0=cps[:, :, 1:65],
                                    in1=free_bcast_ap(rcp[:, :, 0], 2, HD),
                                    op=mybir.AluOpType.mult)

        def build_mlp_thunks(half, tp_ps, mm_ps, blk_mm):
            pre = []          # before the gelu block
            ctxT8h = attn.tile([P, KP, 2, QW], F8, name=f"ctxT8_{half}", tag="ctxT8")
            ln2_t = attn.tile([P, 2, DIM], BF, name=f"ln2_{half}", tag="ln2")
            ln2T = attn.tile([P, KP, 2, QW], F8, name=f"ln2T_{half}", tag="ln2T")
            h1T = mlp.tile([P, 32, QW], BF, name=f"h1T_{half}", tag="h1T")

            def tp_ctx(qt):
                def f():
                    pt = tp_ps.tile([P, KP, 2, P], BF, name="tpt", tag="tp")
                    for kc in range(KC):
                        tpb(pt[:, kc // 2, kc % 2, :],
                            ctxq[half][:, qt, kc * P:(kc + 1) * P],
                            start=(kc == 0), stop=(kc == KC - 1))
                    nc.vector.tensor_copy(ctxT8h[:, :, :, qt * P:(qt + 1) * P], pt[:])
                return f
            for qt in range(2):
                pre.append(tp_ctx(qt))

            op_ps = {}

            def op_mm(mc):
                def f():
                    ps = mm_ps.tile([P, 512], F32, name="ops", tag="mm")
                    op_ps[mc] = ps
                    for kp in range(KP):
                        nc.tensor.matmul(ps[:, 0:QW], wo_sb[:, kp, :, mc, :],
                                         ctxT8h[:, kp, :, :],
                                         start=(kp == 0), stop=(kp == KP - 1),
                                         perf_mode=DR)
                return f

            def op_post(mc):
                def f():
                    ps = op_ps.pop(mc)
                    yt = mlw.tile([P, QW], BF, name="yt", tag="yt")
                    nc.vector.tensor_scalar(out=yt[:], in0=ps[:, 0:QW],
                                            scalar1=OD, scalar2=bo_s[:, mc:mc + 1],
                                            op0=mybir.AluOpType.mult,
                                            op1=mybir.AluOpType.add)
                    pt = tp_ps.tile([P, KP, 2, P], BF, name="tpt", tag="tp")
                    for qt in range(2):
                        tpb(pt[:, 0, qt, :], yt[:, qt * P:(qt + 1) * P],
                            start=(qt == 0), stop=(qt == 1))
                    nc.vector.tensor_tensor(
                        out=tgt2[:, half * 2:half * 2 + 2, mc * P:(mc + 1) * P],
                        in0=pt[:, 0, :, :],
                        in1=tgt_sb[:, half * 2:half * 2 + 2, mc * P:(mc + 1) * P],
                        op=mybir.AluOpType.add)
                return f
            # stagger mm/post so PE is never head-blocked on the DVE tail
            pre.append(op_mm(0))
            for mc in range(1, 8):
                pre.append(op_mm(mc))
                pre.append(op_post(mc - 1))
            pre.append(op_post(7))

            mv2 = mlw.tile([P, 2, nc.vector.BN_AGGR_DIM], F32,
                           name=f"mv2_{half}", tag="mv2")
            rstd2 = mlw.tile([P, 2], F32, name=f"rstd2_{half}", tag="rstd2")

            def ln2_stats(qt):
                def f():
                    gt = half * 2 + qt
                    for sg in range(2):
                        nc.vector.bn_stats(out=st2[:, gt, sg, :],
                                           in_=tgt2[:, gt, sg * 512:(sg + 1) * 512])
                    nc.vector.bn_aggr(out=mv2[:, qt, :], in_=st2[:, gt, :, :])
                return f

            def ln2_rstd():
                # one Ln + one Exp for both query sub-halves
                lnv = mlw.tile([P, 2], F32, name="lnv2", tag="lnv2")
                nc.scalar.activation(out=lnv[:], in_=mv2[:, :, 1], func=AF.Ln,
                                     bias=eps_t[:], scale=1.0)
                nc.scalar.activation(out=rstd2[:], in_=lnv[:], func=AF.Exp,
                                     scale=-0.5)

            def ln2_tp(qt):
                def f():
                    gt = half * 2 + qt
                    nc.vector.tensor_scalar(out=ln2_t[:, qt, :], in0=tgt2[:, gt, :],
                                            scalar1=mv2[:, qt, 0:1],
                                            scalar2=rstd2[:, qt:qt + 1],
                                            op0=mybir.AluOpType.subtract,
                                            op1=mybir.AluOpType.mult)
                    pt = tp_ps.tile([P, KP, 2, P], BF, name="tpt", tag="tp")
                    for kc in range(KC):
                        tpb(pt[:, kc // 2, kc % 2, :],
                            ln2_t[:, qt, kc * P:(kc + 1) * P],
                            start=(kc == 0), stop=(kc == KC - 1))
                    nc.vector.tensor_copy(ln2T[:, :, :, qt * P:(qt + 1) * P], pt[:])
                return f
            pre.append(ln2_stats(0))
            pre.append(ln2_stats(1))
            pre.append(ln2_rstd)
            pre.append(ln2_tp(0))
            pre.append(ln2_tp(1))

            def fc1(hm):
                def f():
                    ps = blk_mm().tile([P, 512], F32, name="f1p", tag="mm")
                    for kp in range(KP):
                        nc.tensor.matmul(ps[:, 0:QW], w1_sb[:, hm, kp, :, :],
                                         ln2T[:, kp, :, :],
                                         start=(kp == 0), stop=(kp == KP - 1),
                                         perf_mode=DR)
                    nc.scalar.activation(h1T[:, hm, :], ps[:, 0:QW], AF.Gelu,
                                         bias=b1_s[:, hm:hm + 1], scale=1.0 / S)
                return f
            gelu_blk = [fc1(hm) for hm in range(32)]

            f2_ps = {}

            def fc2_mm(mc):
                def f():
                    if mc == 0:
                        w2_load(0)
                    if mc + 1 < 8:
                        w2_load(mc + 1)
                    wt = w2_tiles.pop(mc)
                    ps = blk_mm().tile([P, 512], F32, name="f2p", tag="mm")
                    f2_ps[mc] = ps
                    for hc in range(32):
                        nc.tensor.matmul(ps[:, 0:QW], wt[:, hc, :],
                                         h1T[:, hc, :],
                                         start=(hc == 0), stop=(hc == 31))
                return f

            def fc2_post(mc):
                def f():
                    ps = f2_ps.pop(mc)
                    y2 = mlw.tile([P, QW], BF, name="y2", tag="y2")
                    nc.vector.tensor_scalar(out=y2[:], in0=ps[:, 0:QW],
                                            scalar1=OD2, scalar2=b2_s[:, mc:mc + 1],
                                            op0=mybir.AluOpType.mult,
                                            op1=mybir.AluOpType.add)
                    pt = tp_ps.tile([P, KP, 2, P], BF, name="tpt", tag="tp")
                    for qt in range(2):
                        tpb(pt[:, 0, qt, :], y2[:, qt * P:(qt + 1) * P],
                            start=(qt == 0), stop=(qt == 1))
                    nc.vector.tensor_tensor(
                        out=tgt2[:, half * 2:half * 2 + 2, mc * P:(mc + 1) * P],
                        in0=pt[:, 0, :, :],
                        in1=tgt2[:, half * 2:half * 2 + 2, mc * P:(mc + 1) * P],
                        op=mybir.AluOpType.add)
                return f
            post = [fc2_mm(0)]
            for mc in range(1, 8):
                post.append(fc2_mm(mc))
                post.append(fc2_post(mc - 1))
            post.append(fc2_post(7))

            def store():
                for qt in range(2):
                    gt = half * 2 + qt
                    nc.sync.dma_start(out=out[:, gt, :], in_=tgt2[:, gt, :])
            post.append(store)
            return pre, gelu_blk, post

        # ---- half 0: front per quarter + heads (ctx lag 1) ----
        done = []
        for q in range(4):
            emit_front(q)
            for hp in range(4):
                emit_scores_exp((q, hp), 0)
                done.append((q, hp))
                if len(done) >= 2:
                    emit_ctx(done[-2], 0)
        emit_ctx(done[-1], 0)
        vpp_cm.__exit__(None, None, None)
        pjp_cm.__exit__(None, None, None)
        front_cm.__exit__(None, None, None)

        tp_ps = S_.enter_context(tc.tile_pool(name="tp_ps", bufs=1, space="PSUM"))
        mm_ps = S_.enter_context(tc.tile_pool(name="mm_ps", bufs=2, space="PSUM"))
        mlp = S_.enter_context(tc.tile_pool(name="mlp", bufs=1))
        w1pool = S_.enter_context(tc.tile_pool(name="w1p", bufs=1))
        w2pool = S_.enter_context(tc.tile_pool(name="w2p", bufs=2))
        mlw = S_.enter_context(tc.tile_pool(name="mlw", bufs=4))

        w1_sb = w1pool.tile([P, 32, KP, 2, P], F8)
        for c in range(4):
            nc.sync.dma_start(out=w1_sb[:, c * 8:(c + 1) * 8], in_=w1d[:, c * 8:(c + 1) * 8])
        w2_tiles = {}

        def w2_load(mc):
            wt = w2pool.tile([P, 32, P], BF, name=f"w2c{mc % 2}", tag="w2c")
            nc.sync.dma_start(out=wt[:], in_=w2d[:, mc, :, :])
            w2_tiles[mc] = wt

        # ---- mlp0 interleaved with half-1 heads ----
        mmt_box = {}

        def blk_mm():
            return mmt_box["pool"]

        pre0, gelu0, post0 = build_mlp_thunks(0, tp_ps, mm_ps, blk_mm)
        npre = (len(pre0) + 15) // 16
        for i, hd in enumerate(heads):
            emit_scores_exp(hd, 1)
            for t in pre0[i * npre:(i + 1) * npre]:
                t()
            if i >= 2:
                emit_ctx(heads[i - 2], 1)
        for t in pre0[16 * npre:]:
            t()
        emit_ctx(heads[14], 1)
        emit_ctx(heads[15], 1)

        # scores/ctx psum done: release the 5 banks and open a deep ring for
        # the fc1->gelu / fc2 pipelines
        cpp_cm.__exit__(None, None, None)
        scp_cm.__exit__(None, None, None)
        mmt_box["pool"] = S_.enter_context(
            tc.tile_pool(name="mmt", bufs=5, space="PSUM"))

        for t in gelu0:
            t()
        for t in post0:
            t()

        # ---- mlp1 tail ----
        pre1, gelu1, post1 = build_mlp_thunks(1, tp_ps, mmt_box["pool"], blk_mm)
        for t in pre1 + gelu1 + post1:
            t()

    nc.compile()
    return nc


def _get_nc():
    if "nc" not in _CACHE:
        _CACHE["nc"] = _build()
    return _CACHE["nc"]


def kernel(tgt, emb_motion, ln_g, ln_b, wq, bq, wk, bk, wv, bv, wo, bo, w1, b1, w2, b2):
    from concourse.bass_utils import run_bass_kernel_spmd
    import ml_dtypes

    nc = _get_nc()
    f = np.ascontiguousarray
    a32 = lambda x: np.asarray(x, np.float32)
    F8 = ml_dtypes.float8_e4m3

    g32, b32 = a32(ln_g), a32(ln_b)
    wq_e = a32(wq) * g32[:, None]
    bq_e = a32(bq) + b32 @ a32(wq)
    w1_e = a32(w1) * g32[:, None]
    b1_e = a32(b1) + b32 @ a32(w1)
    bo_e = a32(bo) + a32(bv) @ a32(wo)   # softmax rows sum to 1 -> bv folds into bo

    Sv = np.float32(S)

    # wq8/wk8: [c,d] -> [p, Q, kp, kt, g, h'*32+r]
    def qk_tile(w):
        arr = np.asarray(w * Sv, F8)
        arr = arr.reshape(4, 2, 128, 4, 4, 2, 32)      # [kp,kt,p, Q,h',g,r]
        arr = arr.transpose(2, 3, 0, 1, 5, 4, 6)       # [p, Q, kp, kt, g, h', r]
        return f(arr.reshape(128, 4, 4, 2, 2, 128))

    wq_t = qk_tile(wq_e)
    wk_t = qk_tile(a32(wk))   # bk is irrelevant: softmax is shift-invariant in q.bk
    wv_t = f(np.asarray(a32(wv) * Sv, F8).reshape(4, 2, 128, 4, 256).transpose(2, 0, 1, 3, 4))
    wo_t = f(np.asarray(a32(wo) * Sv, F8).reshape(4, 2, 128, 8, 128).transpose(2, 0, 1, 3, 4))
    # w1: [c,hid] -> [p, hm(32), kp(4), 2, 128]
    w1_t = f(np.asarray(w1_e * Sv, F8).reshape(4, 2, 128, 32, 128).transpose(2, 3, 0, 1, 4))
    # w2: [hc*128+p, d] -> [p, mc, hc, 128] bf16
    import ml_dtypes as _md
    w2_t = f(np.asarray(a32(w2), _md.bfloat16).reshape(32, 128, 8, 128).transpose(1, 2, 0, 3))

    def qk_bias(b):
        r = (b * Sv).reshape(4, 4, 2, 32)              # [Q,h',g,r]
        return r.transpose(0, 2, 1, 3).reshape(8, 128).T
    bias_pack = np.concatenate([
        qk_bias(bq_e),
        bo_e.reshape(8, 128).T, a32(b2).reshape(8, 128).T,
        b1_e.reshape(32, 128).T,
    ], axis=1)
    bias_pack = f(bias_pack.astype(np.float32))

    B = tgt.shape[0]
    in_maps = []
    for c in range(8):
        b, h = divmod(c, 2)
        tgt_c = a32(tgt[b, h * NQ:(h + 1) * NQ])
        tgt_t = f(tgt_c.reshape(4, 128, DIM).transpose(1, 0, 2))
        emb_t = np.asarray(a32(emb_motion[b]).T, F8)
        emb_t = f(emb_t.reshape(4, 2, 128, NK).transpose(2, 0, 1, 3))
        in_maps.append({
            "tgt": tgt_t, "emb8": emb_t,
            "wq8": wq_t, "wk8": wk_t, "wv8": wv_t, "wo8": wo_t,
            "w1": w1_t, "w2": w2_t,
            "bias_pack": bias_pack,
        })
    r = run_bass_kernel_spmd(nc, in_maps, list(range(8)))
    res = np.empty((B, 1024, DIM), np.float32)
    for c in range(8):
        b, h = divmod(c, 2)
        res[b, h * NQ:(h + 1) * NQ] = r.results[c]["out"].transpose(1, 0, 2).reshape(NQ, DIM)
    return res
